# revision 3
# baseline (speedup 1.0000x reference)
"""Multi-head "channel attention" kernel for Trainium2 (8 NeuronCores).

Reference computation (B=16, D=512, N=2048, h=8 heads, Nh=256):
    q = Wq @ XQ ; k = Wk @ XK ; v = Wv @ XV          (per batch, (D,N))
    per head (N split into 8 chunks of 256):
      scores = q_h @ k_h^T / sqrt(Nh)                ((D,D), contract over Nh)
      p      = softmax(scores, axis=-1)
      o_h    = p @ v_h                               ((D,Nh), contract over D)
    attn = concat(o_h) ; out = Wo @ (XQ - attn)

Sharding: data-parallel over batch: 16 batches / 8 cores = 2 per core.
No collectives needed.

Per-core kernel strategy (v2):
  * Host passes W.T for all four weights so every matmul operand loads
    from DRAM in its natural layout; the OUTPUT is stored n-major
    ([B, N, D], i.e. out^T per batch) and transposed back on the host.
  * Startup: whole-tensor DMAs issued on the in-order SP queue in
    priority order (wq, xq0, wk, xk0, wv, xv0, head-1 x, wo).  Per-queue
    FIFO makes issue order an effective bandwidth priority, so the first
    QT matmul's inputs (1.5 MB) drain at full rate (~4.2us) instead of
    being interleaved with everything else.  PE warmup matmuls hold the
    HAM clock ramp during that window.
  * Per head: QT, KT, V(et=0), scoresT+exp, V(et=1..3), O-loop.  The
    V(0) group hides the KT PSUM->SBUF copy latency; V(1..3) hide the
    last exp; so the PE never waits on Scalar/Vector between phases.
      QT = XQ^T @ Wq^T  -> lhsT = XQ tile, rhs = WqT   (n-major)
      V  = Wv  @ XV     -> lhsT = WvT tile, rhs = XV   (d-major)
      scoresT (e-part, d-free) = lhsT(KT) x rhs(QT); exp straight out
      of PSUM with scale 1/16 (scores ~N(0,1): no overflow, so no max
      subtraction).
  * O-matmul: lhsT = exp(scoresT) tiles, rhs = V with two extra columns
    fixed to -1.0, so PSUM column 256 accumulates -sum_e(p) = -r (col
    257 pads the fp32r dst free count to even).  reciprocal gives -1/r
    and one fused scalar_tensor_tensor does Z = XQ + O * (-1/r).
  * Output projection is n-major: outT[n, e] = sum_d Z[d, n] WoT[d, e],
    accumulated INCREMENTALLY in two held PSUM banks as each Z d-tile
    is produced by the O-loop (one dt behind, so the PE never waits on
    the Z writes).  After the loop only one copy + one 256-row DMA per
    128-row block remains -> no end-of-kernel backlog.  Head 0's outT
    is deferred into head 1's O-loop because WoT lands late in the
    startup DMA priority order.
  * All matmul operands are float32r (TF32-like): 1 cycle/row on the PE
    when the moving free dim >= 256.
"""

import sys

if "/opt/trn_rl_repo" not in sys.path:
    sys.path.insert(0, "/opt/trn_rl_repo")

import numpy as np

import concourse.bass as bass
import concourse.tile as tile
from concourse import bacc, mybir
from concourse.bass_utils import run_bass_kernel_spmd

B_PER_CORE = 2
D = 512
N = 2048
H = 8
NH = N // H  # 256
PT = D // 128  # 4 partition tiles over D
HT = NH // 128  # 2 partition tiles over one head's n-range

F32 = mybir.dt.float32
F32R = mybir.dt.float32r

_NC_CACHE = None


def build_nc():
    nc = bacc.Bacc("TRN2", target_bir_lowering=False, debug=False)

    xq = nc.dram_tensor("xq", [B_PER_CORE, D, N], F32R, kind="ExternalInput").ap()
    xk = nc.dram_tensor("xk", [B_PER_CORE, D, N], F32R, kind="ExternalInput").ap()
    xv = nc.dram_tensor("xv", [B_PER_CORE, D, N], F32R, kind="ExternalInput").ap()
    wqt = nc.dram_tensor("wqt", [D, D], F32R, kind="ExternalInput").ap()
    wkt = nc.dram_tensor("wkt", [D, D], F32R, kind="ExternalInput").ap()
    wvt = nc.dram_tensor("wvt", [D, D], F32R, kind="ExternalInput").ap()
    wot = nc.dram_tensor("wot", [D, D], F32R, kind="ExternalInput").ap()
    # n-major output: out[b, n, e] = final[b, e, n]; host transposes back.
    out = nc.dram_tensor("out", [B_PER_CORE, N, D], F32, kind="ExternalOutput").ap()

    with tile.TileContext(nc) as tc:
        with (
            tc.tile_pool(name="wpool", bufs=1) as wpool,
            tc.tile_pool(name="zpool", bufs=3) as zpool,
            tc.tile_pool(name="xpool", bufs=3) as xpool,
            tc.tile_pool(name="qkpool", bufs=2) as qkpool,
            tc.tile_pool(name="vpool", bufs=2) as vpool,
            tc.tile_pool(name="ptpool", bufs=2) as ptpool,
            tc.tile_pool(name="opool", bufs=6) as opool,
            tc.tile_pool(name="rpool", bufs=6) as rpool,
            tc.tile_pool(name="psq", bufs=2, space="PSUM") as psq,
            tc.tile_pool(name="pss", bufs=2, space="PSUM") as pss,
            tc.tile_pool(name="pso", bufs=2, space="PSUM") as pso,
            tc.tile_pool(name="pst", bufs=1, space="PSUM") as pst,
        ):
            # Weights resident for the whole kernel: [p, it, o] = W.T[it*128+p, o]
            w_sb = {}
            w_dram = {"wq": wqt, "wk": wkt, "wv": wvt, "wo": wot}

            def load_w(name):
                w_sb[name] = wpool.tile(
                    [128, PT, D], F32R, name=f"w_{name}", tag=f"w_{name}"
                )
                src = w_dram[name].rearrange("(t p) o -> p t o", p=128)
                nc.sync.dma_start(out=w_sb[name], in_=src)

            x_b = {
                "xq": [xq[b].rearrange("(t p) n -> p t n", p=128) for b in range(B_PER_CORE)],
                "xk": [xk[b].rearrange("(t p) n -> p t n", p=128) for b in range(B_PER_CORE)],
                "xv": [xv[b].rearrange("(t p) n -> p t n", p=128) for b in range(B_PER_CORE)],
            }

            def load_x(b, h, nm):
                ns_ = slice(h * NH, (h + 1) * NH)
                t = xpool.tile([128, PT, NH], F32R, name=f"{nm}_h", tag=f"{nm}_h")
                nc.sync.dma_start(out=t, in_=x_b[nm][b][:, :, ns_])
                return t

            def load_head(b, h):
                return [load_x(b, h, nm) for nm in ("xq", "xk", "xv")]

            steps = [(b, h) for b in range(B_PER_CORE) for h in range(H)]
            head_tiles = {}
            # head 0's (z_h, b, h) awaiting deferred outT during head 1
            deferred = []

            warm = None
            ps_warm = None

            def warmup(n):
                for _ in range(n):
                    ps_w = psq.tile([128, D], F32, name="ps_p", tag="ps_p")
                    nc.tensor.matmul(
                        ps_w, lhsT=warm[:, 0:128], rhs=warm, start=True, stop=True
                    )

            def store_outT(b, h, jn, src_ps):
                """Copy one accumulated outT block to SBUF and DMA it out."""
                o_sb = opool.tile([128, D], F32, name="o_sb", tag="o_sb")
                nc.vector.tensor_copy(out=o_sb, in_=src_ps)
                n0 = h * NH + jn * 128
                nc.sync.dma_start(out=out[b, n0 : n0 + 128, :], in_=o_sb)

            def emit_deferred_group(b, h, z_h, jn):
                """One full-Z n-major output-projection group (head 0 path)."""
                ps = psq.tile([128, D], F32, name="ps_p", tag="ps_p")
                for it in range(PT):
                    nc.tensor.matmul(
                        ps,
                        lhsT=z_h[:, it, jn * 128 : (jn + 1) * 128],
                        rhs=w_sb["wo"][:, it, :],
                        start=(it == 0),
                        stop=(it == PT - 1),
                    )
                store_outT(b, h, jn, ps)

            for idx, (b, h) in enumerate(steps):
                ns = slice(h * NH, (h + 1) * NH)

                if idx == 0:
                    # PE warmup: matmuls on dummy data during the initial DMA
                    # window hold the HAM clock ramp (otherwise the first
                    # ~3.4us run at 1.2GHz and the ramp restarts on idle).
                    warm = wpool.tile([128, D], F32R, name="warm", tag="warm")
                    nc.scalar.activation(
                        out=warm,
                        in_=warm.bitcast(F32),
                        func=mybir.ActivationFunctionType.Copy,
                        bias=0.0,
                        scale=0.0,
                    )
                    warmup(7)
                    # Priority-ordered whole-tensor startup DMAs: the SP
                    # queue is in-order and per-DMA-queue FIFO, so earlier
                    # issues get full bandwidth until they complete.
                    load_w("wq")
                    t_xq = load_x(0, 0, "xq")
                    load_w("wk")
                    t_xk = load_x(0, 0, "xk")
                    load_w("wv")
                    t_xv = load_x(0, 0, "xv")
                    head_tiles[(0, 1)] = load_head(0, 1)
                    load_w("wo")
                    head_tiles[(0, 0)] = [t_xq, t_xk, t_xv]

                xq_h, xk_h, xv_h = head_tiles.pop((b, h))
                # Prefetch the next head's inputs (heads 0 and 1 were
                # prefetched in the startup sequence).
                if idx >= 1 and idx + 1 < len(steps):
                    head_tiles[steps[idx + 1]] = load_head(*steps[idx + 1])

                # QT/KT: [p, jt, d] = X^T @ W^T  (n-major projections)
                qt_h = qkpool.tile([128, HT, D], F32R, name="qt_h", tag="qt_h")
                kt_h = qkpool.tile([128, HT, D], F32R, name="kt_h", tag="kt_h")
                for dst, src, w in ((qt_h, xq_h, "wq"), (kt_h, xk_h, "wk")):
                    for jt in range(HT):
                        ps = psq.tile([128, D], F32, name="ps_p", tag="ps_p")
                        for it in range(PT):
                            nc.tensor.matmul(
                                ps,
                                lhsT=src[:, it, jt * 128 : (jt + 1) * 128],
                                rhs=w_sb[w][:, it, :],
                                start=(it == 0),
                                stop=(it == PT - 1),
                            )
                        nc.scalar.copy(out=dst[:, jt, :], in_=ps)
                    if idx == 0 and dst is qt_h:
                        # wk/xk still in flight during head 0's QT.
                        warmup(1)

                # V (d-major): [p, et, n]; columns NH/NH+1 fixed at -1.0 so
                # the O-matmul accumulates -r in PSUM column NH (col NH+1
                # is padding: fp32r matmul dst free count must be even).
                v_h = vpool.tile([128, PT, NH + 2], F32R, name="v_h", tag="v_h")
                # memset can't emit fp32r; ACT Copy(in*0 - 1) = -1.0 can.
                nc.scalar.activation(
                    out=v_h[:, :, NH : NH + 2],
                    in_=w_sb["wv"][:, :, 0:2],
                    func=mybir.ActivationFunctionType.Copy,
                    bias=-1.0,
                    scale=0.0,
                )

                def v_group(et):
                    ps = psq.tile([128, D], F32, name="ps_p", tag="ps_p")
                    for it in range(PT):
                        nc.tensor.matmul(
                            ps[:, 0:NH],
                            lhsT=w_sb["wv"][:, it, et * 128 : (et + 1) * 128],
                            rhs=xv_h[:, it, :],
                            start=(it == 0),
                            stop=(it == PT - 1),
                        )
                    nc.vector.tensor_copy(out=v_h[:, et, 0:NH], in_=ps[:, 0:NH])

                # Head 0: V data lands after scores' inputs, so run all of V
                # after scores (covered by warmups).  Steady state: V(0)
                # between KT and scores hides the KT copy; V(1..3) after
                # scores hide the last exp.
                if idx >= 1:
                    v_group(0)

                # scoresT (e-part, d-free) then p~ = exp(scoresT / 16)
                pt_t = ptpool.tile([128, PT, D], F32R, name="pt_t", tag="pt_t")
                for et in range(PT):
                    ps_s = pss.tile([128, D], F32, name="ps_s", tag="ps_s")
                    for jt in range(HT):
                        nc.tensor.matmul(
                            ps_s,
                            lhsT=kt_h[:, jt, et * 128 : (et + 1) * 128],
                            rhs=qt_h[:, jt, :],
                            start=(jt == 0),
                            stop=(jt == HT - 1),
                        )
                    nc.scalar.activation(
                        out=pt_t[:, et, :],
                        in_=ps_s,
                        func=mybir.ActivationFunctionType.Exp,
                        scale=float(1.0 / np.sqrt(NH)),
                    )

                if idx == 0:
                    warmup(2)
                    for et in range(PT):
                        v_group(et)
                else:
                    for et in range(1, PT):
                        v_group(et)

                # O = p~ @ [V | -1 | -1]; col NH = -r; Z = XQ + O * (-1/r).
                # outT accumulates one dt behind in two held PSUM banks:
                #   outT[jn][n, e] += Z[d, n] WoT[d, e]   (d = dt tile)
                # so after the last O group only copies + DMAs remain.
                z_h = zpool.tile([128, PT, NH], F32R, name="z_h", tag="z_h")
                incr = idx >= 1
                if incr:
                    psT = [
                        pst.tile([128, D], F32, name=f"ps_t{jn}", tag=f"ps_t{jn}")
                        for jn in range(HT)
                    ]

                def outT_mm(dt_, stop):
                    for jn in range(HT):
                        nc.tensor.matmul(
                            psT[jn],
                            lhsT=z_h[:, dt_, jn * 128 : (jn + 1) * 128],
                            rhs=w_sb["wo"][:, dt_, :],
                            start=(dt_ == 0),
                            stop=stop,
                        )

                for dt_ in range(PT):
                    ps_o = pso.tile([128, NH + 2], F32, name="ps_o", tag="ps_o")
                    for et in range(PT):
                        nc.tensor.matmul(
                            ps_o,
                            lhsT=pt_t[:, et, dt_ * 128 : (dt_ + 1) * 128],
                            rhs=v_h[:, et, :],
                            start=(et == 0),
                            stop=(et == PT - 1),
                        )
                    recip = rpool.tile([128, 1], F32, name="recip", tag="recip")
                    nc.vector.reciprocal(recip, ps_o[:, NH : NH + 1])
                    nc.vector.scalar_tensor_tensor(
                        out=z_h[:, dt_, :],
                        in0=ps_o[:, 0:NH],
                        scalar=recip,
                        in1=xq_h[:, dt_, :].bitcast(F32),
                        op0=mybir.AluOpType.mult,
                        op1=mybir.AluOpType.add,
                    )
                    if incr and dt_ >= 1:
                        outT_mm(dt_ - 1, stop=False)
                    if deferred and idx == 1 and dt_ >= 2:
                        db, dh, dz = deferred[0]
                        emit_deferred_group(db, dh, dz, dt_ - 2)

                if incr:
                    outT_mm(PT - 1, stop=True)
                    for jn in range(HT):
                        store_outT(b, h, jn, psT[jn])
                else:
                    deferred.append((b, h, z_h))

    nc.compile()
    return nc


def _get_nc():
    global _NC_CACHE
    if _NC_CACHE is None:
        _NC_CACHE = build_nc()
    return _NC_CACHE


def _shard_inputs(inputs):
    xq = np.ascontiguousarray(np.asarray(inputs["X_Query"], dtype=np.float32))
    xk = np.ascontiguousarray(np.asarray(inputs["X_Key"], dtype=np.float32))
    xv = np.ascontiguousarray(np.asarray(inputs["X_Value"], dtype=np.float32))
    weights = {
        "wqt": np.ascontiguousarray(np.asarray(inputs["W_q"], dtype=np.float32).T),
        "wkt": np.ascontiguousarray(np.asarray(inputs["W_k"], dtype=np.float32).T),
        "wvt": np.ascontiguousarray(np.asarray(inputs["W_v"], dtype=np.float32).T),
        "wot": np.ascontiguousarray(np.asarray(inputs["W_o"], dtype=np.float32).T),
    }
    in_maps = []
    for c in range(8):
        sl = slice(c * B_PER_CORE, (c + 1) * B_PER_CORE)
        in_maps.append(
            {"xq": xq[sl], "xk": xk[sl], "xv": xv[sl], **weights}
        )
    return in_maps


def run_sharded(inputs, **kwargs):
    """Run on all 8 cores; returns (full_output, BassKernelResults)."""
    nc = _get_nc()
    in_maps = _shard_inputs(inputs)
    res = run_bass_kernel_spmd(nc, in_maps, core_ids=list(range(8)), **kwargs)
    # per-core out is [B_PER_CORE, N, D] (n-major); transpose back.
    full = np.concatenate(
        [np.ascontiguousarray(r["out"].transpose(0, 2, 1)) for r in res.results],
        axis=0,
    )
    return full, res


def kernel(**inputs):
    full, _ = run_sharded(inputs)
    return full


# revision 7
# speedup vs baseline: 1.0059x; 1.0059x over previous
"""Multi-head "channel attention" kernel for Trainium2 (8 NeuronCores).

Reference computation (B=16, D=512, N=2048, h=8 heads, Nh=256):
    q = Wq @ XQ ; k = Wk @ XK ; v = Wv @ XV          (per batch, (D,N))
    per head (N split into 8 chunks of 256):
      scores = q_h @ k_h^T / sqrt(Nh)                ((D,D), contract over Nh)
      p      = softmax(scores, axis=-1)
      o_h    = p @ v_h                               ((D,Nh), contract over D)
    attn = concat(o_h) ; out = Wo @ (XQ - attn)

Sharding: data-parallel over batch: 16 batches / 8 cores = 2 per core.
No collectives needed.

Per-core kernel strategy (v2):
  * Host passes W.T for all four weights so every matmul operand loads
    from DRAM in its natural layout; the OUTPUT is stored n-major
    ([B, N, D], i.e. out^T per batch) and transposed back on the host.
  * Startup: whole-tensor DMAs issued on the in-order SP queue in
    priority order (wq, xq0, wk, xk0, wv, xv0, head-1 x, wo).  Per-queue
    FIFO makes issue order an effective bandwidth priority, so the first
    QT matmul's inputs (1.5 MB) drain at full rate (~4.2us) instead of
    being interleaved with everything else.  PE warmup matmuls hold the
    HAM clock ramp during that window.
  * Per head: QT, KT, V(et=0), scoresT+exp, V(et=1..3), O-loop.  The
    V(0) group hides the KT PSUM->SBUF copy latency; V(1..3) hide the
    last exp; so the PE never waits on Scalar/Vector between phases.
      QT = XQ^T @ Wq^T  -> lhsT = XQ tile, rhs = WqT   (n-major)
      V  = Wv  @ XV     -> lhsT = WvT tile, rhs = XV   (d-major)
      scoresT (e-part, d-free) = lhsT(KT) x rhs(QT); exp straight out
      of PSUM with scale 1/16 (scores ~N(0,1): no overflow, so no max
      subtraction).
  * O-matmul: lhsT = exp(scoresT) tiles, rhs = V with two extra columns
    fixed to -1.0, so PSUM column 256 accumulates -sum_e(p) = -r (col
    257 pads the fp32r dst free count to even).  reciprocal gives -1/r
    and one fused scalar_tensor_tensor does Z = XQ + O * (-1/r).
  * Output projection is n-major: outT[n, e] = sum_d Z[d, n] WoT[d, e],
    accumulated INCREMENTALLY in two held PSUM banks as each Z d-tile
    is produced by the O-loop (one dt behind, so the PE never waits on
    the Z writes).  After the loop only one copy + one 256-row DMA per
    128-row block remains -> no end-of-kernel backlog.  Head 0's outT
    is deferred into head 1's O-loop because WoT lands late in the
    startup DMA priority order.
  * All matmul operands are float32r (TF32-like): 1 cycle/row on the PE
    when the moving free dim >= 256.
"""

import sys

if "/opt/trn_rl_repo" not in sys.path:
    sys.path.insert(0, "/opt/trn_rl_repo")

import numpy as np

import concourse.bass as bass
import concourse.tile as tile
from concourse import bacc, mybir
from concourse.bass_utils import run_bass_kernel_spmd

B_PER_CORE = 2
D = 512
N = 2048
H = 8
NH = N // H  # 256
PT = D // 128  # 4 partition tiles over D
HT = NH // 128  # 2 partition tiles over one head's n-range

F32 = mybir.dt.float32
F32R = mybir.dt.float32r

_NC_CACHE = None


def build_nc():
    nc = bacc.Bacc("TRN2", target_bir_lowering=False, debug=False)

    xq = nc.dram_tensor("xq", [B_PER_CORE, D, N], F32R, kind="ExternalInput").ap()
    xk = nc.dram_tensor("xk", [B_PER_CORE, D, N], F32R, kind="ExternalInput").ap()
    xv = nc.dram_tensor("xv", [B_PER_CORE, D, N], F32R, kind="ExternalInput").ap()
    wqt = nc.dram_tensor("wqt", [D, D], F32R, kind="ExternalInput").ap()
    wkt = nc.dram_tensor("wkt", [D, D], F32R, kind="ExternalInput").ap()
    wvt = nc.dram_tensor("wvt", [D, D], F32R, kind="ExternalInput").ap()
    wot = nc.dram_tensor("wot", [D, D], F32R, kind="ExternalInput").ap()
    # n-major output: out[b, n, e] = final[b, e, n]; host transposes back.
    out = nc.dram_tensor("out", [B_PER_CORE, N, D], F32, kind="ExternalOutput").ap()

    with tile.TileContext(nc) as tc:
        with (
            tc.tile_pool(name="wpool", bufs=1) as wpool,
            tc.tile_pool(name="zpool", bufs=3) as zpool,
            tc.tile_pool(name="xpool", bufs=3) as xpool,
            tc.tile_pool(name="qkpool", bufs=2) as qkpool,
            tc.tile_pool(name="vpool", bufs=2) as vpool,
            tc.tile_pool(name="ptpool", bufs=2) as ptpool,
            tc.tile_pool(name="opool", bufs=6) as opool,
            tc.tile_pool(name="rpool", bufs=6) as rpool,
            tc.tile_pool(name="psq", bufs=2, space="PSUM") as psq,
            tc.tile_pool(name="pss", bufs=2, space="PSUM") as pss,
            tc.tile_pool(name="pso", bufs=2, space="PSUM") as pso,
            tc.tile_pool(name="pst", bufs=1, space="PSUM") as pst,
        ):
            # Weights resident for the whole kernel: [p, it, o] = W.T[it*128+p, o]
            w_sb = {}
            w_dram = {"wq": wqt, "wk": wkt, "wv": wvt, "wo": wot}

            def load_w(name, parts=1):
                # parts>1 splits the load into multiple dma_starts: a single
                # dma_start's descriptors only spread over ~4-6 of the 16 DMA
                # queues, so chunking raises effective transfer bandwidth.
                w_sb[name] = wpool.tile(
                    [128, PT, D], F32R, name=f"w_{name}", tag=f"w_{name}"
                )
                src = w_dram[name].rearrange("(t p) o -> p t o", p=128)
                step = PT // parts
                for c in range(parts):
                    sl = slice(c * step, (c + 1) * step)
                    nc.sync.dma_start(out=w_sb[name][:, sl, :], in_=src[:, sl, :])

            x_b = {
                "xq": [xq[b].rearrange("(t p) n -> p t n", p=128) for b in range(B_PER_CORE)],
                "xk": [xk[b].rearrange("(t p) n -> p t n", p=128) for b in range(B_PER_CORE)],
                "xv": [xv[b].rearrange("(t p) n -> p t n", p=128) for b in range(B_PER_CORE)],
            }

            def load_x(b, h, nm, parts=1):
                ns_ = slice(h * NH, (h + 1) * NH)
                t = xpool.tile([128, PT, NH], F32R, name=f"{nm}_h", tag=f"{nm}_h")
                step = PT // parts
                for c in range(parts):
                    sl = slice(c * step, (c + 1) * step)
                    nc.sync.dma_start(
                        out=t[:, sl, :], in_=x_b[nm][b][:, sl, ns_]
                    )
                return t

            def load_head(b, h):
                return [load_x(b, h, nm) for nm in ("xq", "xk", "xv")]

            steps = [(b, h) for b in range(B_PER_CORE) for h in range(H)]
            head_tiles = {}
            # head 0's (z_h, b, h) awaiting deferred outT during head 1
            deferred = []

            warm = None
            ps_warm = None

            def warmup(n):
                for _ in range(n):
                    ps_w = psq.tile([128, D], F32, name="ps_p", tag="ps_p")
                    nc.tensor.matmul(
                        ps_w, lhsT=warm[:, 0:128], rhs=warm, start=True, stop=True
                    )

            def store_outT(b, h, jn, src_ps):
                """Copy one accumulated outT block to SBUF and DMA it out."""
                o_sb = opool.tile([128, D], F32, name="o_sb", tag="o_sb")
                nc.vector.tensor_copy(out=o_sb, in_=src_ps)
                n0 = h * NH + jn * 128
                nc.sync.dma_start(out=out[b, n0 : n0 + 128, :], in_=o_sb)

            def emit_deferred_group(b, h, z_h, jn):
                """One full-Z n-major output-projection group (head 0 path)."""
                ps = psq.tile([128, D], F32, name="ps_p", tag="ps_p")
                for it in range(PT):
                    nc.tensor.matmul(
                        ps,
                        lhsT=z_h[:, it, jn * 128 : (jn + 1) * 128],
                        rhs=w_sb["wo"][:, it, :],
                        start=(it == 0),
                        stop=(it == PT - 1),
                    )
                store_outT(b, h, jn, ps)

            for idx, (b, h) in enumerate(steps):
                ns = slice(h * NH, (h + 1) * NH)

                if idx == 0:
                    # PE warmup: matmuls on dummy data during the initial DMA
                    # window hold the HAM clock ramp (otherwise the first
                    # ~3.4us run at 1.2GHz and the ramp restarts on idle).
                    warm = wpool.tile([128, D], F32R, name="warm", tag="warm")
                    nc.scalar.activation(
                        out=warm,
                        in_=warm.bitcast(F32),
                        func=mybir.ActivationFunctionType.Copy,
                        bias=0.0,
                        scale=0.0,
                    )
                    warmup(7)
                    # Priority-ordered chunked startup DMAs: the SP queue is
                    # in-order and the DMA queues are FIFO, so issue order is
                    # an effective bandwidth priority.  The first pair
                    # (wq+xq) is chunked finest to light up all 16 queues.
                    w_sb["wq"] = wpool.tile(
                        [128, PT, D], F32R, name="w_wq", tag="w_wq"
                    )
                    wq_src = w_dram["wq"].rearrange("(t p) o -> p t o", p=128)
                    t_xq = xpool.tile([128, PT, NH], F32R, name="xq_h", tag="xq_h")
                    for it in range(PT):
                        nc.sync.dma_start(
                            out=w_sb["wq"][:, it : it + 1, :],
                            in_=wq_src[:, it : it + 1, :],
                        )
                        nc.sync.dma_start(
                            out=t_xq[:, it : it + 1, :],
                            in_=x_b["xq"][0][:, it : it + 1, ns],
                        )
                    load_w("wk", parts=2)
                    t_xk = load_x(0, 0, "xk", parts=2)
                    load_w("wv", parts=2)
                    t_xv = load_x(0, 0, "xv", parts=2)
                    head_tiles[(0, 1)] = load_head(0, 1)
                    load_w("wo", parts=2)
                    head_tiles[(0, 0)] = [t_xq, t_xk, t_xv]

                xq_h, xk_h, xv_h = head_tiles.pop((b, h))
                # Prefetch the next head's inputs (heads 0 and 1 were
                # prefetched in the startup sequence).
                if idx >= 1 and idx + 1 < len(steps):
                    head_tiles[steps[idx + 1]] = load_head(*steps[idx + 1])

                # QT/KT: [p, jt, d] = X^T @ W^T  (n-major projections)
                qt_h = qkpool.tile([128, HT, D], F32R, name="qt_h", tag="qt_h")
                kt_h = qkpool.tile([128, HT, D], F32R, name="kt_h", tag="kt_h")
                for dst, src, w in ((qt_h, xq_h, "wq"), (kt_h, xk_h, "wk")):
                    for jt in range(HT):
                        ps = psq.tile([128, D], F32, name="ps_p", tag="ps_p")
                        for it in range(PT):
                            nc.tensor.matmul(
                                ps,
                                lhsT=src[:, it, jt * 128 : (jt + 1) * 128],
                                rhs=w_sb[w][:, it, :],
                                start=(it == 0),
                                stop=(it == PT - 1),
                            )
                        nc.scalar.copy(out=dst[:, jt, :], in_=ps)
                    if idx == 0 and dst is qt_h:
                        # wk/xk still in flight during head 0's QT.
                        warmup(2)

                # V (d-major): [p, et, n]; columns NH/NH+1 fixed at -1.0 so
                # the O-matmul accumulates -r in PSUM column NH (col NH+1
                # is padding: fp32r matmul dst free count must be even).
                v_h = vpool.tile([128, PT, NH + 2], F32R, name="v_h", tag="v_h")
                # memset can't emit fp32r; ACT Copy(in*0 - 1) = -1.0 can.
                nc.scalar.activation(
                    out=v_h[:, :, NH : NH + 2],
                    in_=w_sb["wv"][:, :, 0:2],
                    func=mybir.ActivationFunctionType.Copy,
                    bias=-1.0,
                    scale=0.0,
                )

                def v_group(et):
                    ps = psq.tile([128, D], F32, name="ps_p", tag="ps_p")
                    for it in range(PT):
                        nc.tensor.matmul(
                            ps[:, 0:NH],
                            lhsT=w_sb["wv"][:, it, et * 128 : (et + 1) * 128],
                            rhs=xv_h[:, it, :],
                            start=(it == 0),
                            stop=(it == PT - 1),
                        )
                    nc.vector.tensor_copy(out=v_h[:, et, 0:NH], in_=ps[:, 0:NH])

                # Head 0: V data lands after scores' inputs, so run all of V
                # after scores (covered by warmups).  Steady state: V(0)
                # between KT and scores hides the KT copy; V(1..3) after
                # scores hide the last exp.
                if idx >= 1:
                    v_group(0)

                # scoresT (e-part, d-free) then p~ = exp(scoresT / 16)
                pt_t = ptpool.tile([128, PT, D], F32R, name="pt_t", tag="pt_t")
                for et in range(PT):
                    ps_s = pss.tile([128, D], F32, name="ps_s", tag="ps_s")
                    for jt in range(HT):
                        nc.tensor.matmul(
                            ps_s,
                            lhsT=kt_h[:, jt, et * 128 : (et + 1) * 128],
                            rhs=qt_h[:, jt, :],
                            start=(jt == 0),
                            stop=(jt == HT - 1),
                        )
                    nc.scalar.activation(
                        out=pt_t[:, et, :],
                        in_=ps_s,
                        func=mybir.ActivationFunctionType.Exp,
                        scale=float(1.0 / np.sqrt(NH)),
                    )

                if idx == 0:
                    warmup(2)
                    for et in range(PT):
                        v_group(et)
                else:
                    for et in range(1, PT):
                        v_group(et)

                # O = p~ @ [V | -1 | -1]; col NH = -r; Z = XQ + O * (-1/r).
                # outT accumulates one dt behind in two held PSUM banks:
                #   outT[jn][n, e] += Z[d, n] WoT[d, e]   (d = dt tile)
                # so after the last O group only copies + DMAs remain.
                z_h = zpool.tile([128, PT, NH], F32R, name="z_h", tag="z_h")
                incr = idx >= 1
                if incr:
                    psT = [
                        pst.tile([128, D], F32, name=f"ps_t{jn}", tag=f"ps_t{jn}")
                        for jn in range(HT)
                    ]

                def outT_mm(dt_, stop):
                    for jn in range(HT):
                        nc.tensor.matmul(
                            psT[jn],
                            lhsT=z_h[:, dt_, jn * 128 : (jn + 1) * 128],
                            rhs=w_sb["wo"][:, dt_, :],
                            start=(dt_ == 0),
                            stop=stop,
                        )

                for dt_ in range(PT):
                    ps_o = pso.tile([128, NH + 2], F32, name="ps_o", tag="ps_o")
                    for et in range(PT):
                        nc.tensor.matmul(
                            ps_o,
                            lhsT=pt_t[:, et, dt_ * 128 : (dt_ + 1) * 128],
                            rhs=v_h[:, et, :],
                            start=(et == 0),
                            stop=(et == PT - 1),
                        )
                    recip = rpool.tile([128, 1], F32, name="recip", tag="recip")
                    nc.vector.reciprocal(recip, ps_o[:, NH : NH + 1])
                    nc.vector.scalar_tensor_tensor(
                        out=z_h[:, dt_, :],
                        in0=ps_o[:, 0:NH],
                        scalar=recip,
                        in1=xq_h[:, dt_, :].bitcast(F32),
                        op0=mybir.AluOpType.mult,
                        op1=mybir.AluOpType.add,
                    )
                    if incr and dt_ >= 1:
                        outT_mm(dt_ - 1, stop=False)
                    if deferred and idx == 1 and dt_ >= 2:
                        db, dh, dz = deferred[0]
                        emit_deferred_group(db, dh, dz, dt_ - 2)

                if incr:
                    outT_mm(PT - 1, stop=True)
                    for jn in range(HT):
                        store_outT(b, h, jn, psT[jn])
                else:
                    deferred.append((b, h, z_h))

    nc.compile()
    return nc


def _get_nc():
    global _NC_CACHE
    if _NC_CACHE is None:
        _NC_CACHE = build_nc()
    return _NC_CACHE


def _shard_inputs(inputs):
    xq = np.ascontiguousarray(np.asarray(inputs["X_Query"], dtype=np.float32))
    xk = np.ascontiguousarray(np.asarray(inputs["X_Key"], dtype=np.float32))
    xv = np.ascontiguousarray(np.asarray(inputs["X_Value"], dtype=np.float32))
    weights = {
        "wqt": np.ascontiguousarray(np.asarray(inputs["W_q"], dtype=np.float32).T),
        "wkt": np.ascontiguousarray(np.asarray(inputs["W_k"], dtype=np.float32).T),
        "wvt": np.ascontiguousarray(np.asarray(inputs["W_v"], dtype=np.float32).T),
        "wot": np.ascontiguousarray(np.asarray(inputs["W_o"], dtype=np.float32).T),
    }
    in_maps = []
    for c in range(8):
        sl = slice(c * B_PER_CORE, (c + 1) * B_PER_CORE)
        in_maps.append(
            {"xq": xq[sl], "xk": xk[sl], "xv": xv[sl], **weights}
        )
    return in_maps


def run_sharded(inputs, **kwargs):
    """Run on all 8 cores; returns (full_output, BassKernelResults)."""
    nc = _get_nc()
    in_maps = _shard_inputs(inputs)
    res = run_bass_kernel_spmd(nc, in_maps, core_ids=list(range(8)), **kwargs)
    # per-core out is [B_PER_CORE, N, D] (n-major); transpose back.
    full = np.concatenate(
        [np.ascontiguousarray(r["out"].transpose(0, 2, 1)) for r in res.results],
        axis=0,
    )
    return full, res


def kernel(**inputs):
    full, _ = run_sharded(inputs)
    return full
